# revision 45
# baseline (speedup 1.0000x reference)
"""MixedQLinear Trainium2 kernel — token-parallel, merged-K version.

Computation (per reference):
  x2 = x[0]                                  (M=4096, IN_F=4096) fp16
  int_x = x2[:, int_indices]                 (M, 3840)
  fp_x  = x2[:, fp_indices]                  (M, 256)
  per-token asym quant of int_x to int4:  scale=(mx-mn)/15, zero=mn
  q = round((int_x-zero)/scale) - 8          in [-8,7]
  out = scale*w_scale*(q @ w_int.T) + (zero+8*scale)*reduced_w + fp_x@fp_w.T + bias

Strategy: shard TOKENS across the 8 cores (512 each); every core holds
the full out_features dimension.  No collective is needed.

Key algebra: divide the fp weights by wscale on the host and the fp
activations by scale_t on the device, then the fp path rides INSIDE the
int matmul (K = 3840 int + 256 fp = 4096 exactly = 16 fp8 DoubleRow
matmuls), because the combine multiplies the whole psum by
scale_t*wscale_o.  The zero-point term (mn+8*scale)*reduced_w becomes
(mn*rs+8) * (reduced_w/wscale) * (scale*wscale) and is pre-filled into
the PSUM bank by the ACT engine before the matmuls accumulate onto it.
Final combine is ONE fused DVE op: out = (p0*scale_t)*wscale_bcast.

Phase A per 128-token tile (split in 2 K-chunks for latency):
  - min (DVE fused tensor_tensor_reduce) / max (Pool) stats,
  - params: scale, rs=1/scale, bqC=-mn*rs-8+C, alphas=mn*rs+8,
  - quant+round in ONE ACT op: y = x*rs + (bqC) in f32 (magic constant
    C=1.5*2^23 forces RNE-to-integer), DVE subtracts C casting to f16,
  - fp cols: ACT copy with scale=rs into the same qa staging,
  - DMA xbar transpose (sync ring) -> k-major, DVE cast f16->f8.

DMA queues: weights (16.8MB) alone on the scalar HWDGE ring; the 8
transposes alone on the sync HWDGE ring; x tiles, consts and output
stores on the gpsimd SWDGE queue.

Host side does only layout work: column gather, int4 unpack, weight
merge + fp8 cast, broadcasts, token slicing, concat of outputs.
"""

import os
import sys

import numpy as np

for _p in ("/opt/trn_rl_repo",):
    if _p not in sys.path and os.path.isdir(_p):
        sys.path.insert(0, _p)

TOKENS = 4096
IN_F = 4096
OUT_F = 4096
FP_F = 256
INT_F = IN_F - FP_F          # 3840
NCORES = 8
TPC = TOKENS // NCORES       # 512 tokens per core
NT = TPC // 128              # 4 token tiles per core
KE = IN_F // 128             # 32 k-planes (30 int + 2 fp)
HKE = KE // 2                # 16 planes per half-chunk
NG = 8                       # out-feature groups per core
OG = OUT_F // NG             # 512 out features per group
C_MAGIC = 12582912.0         # 1.5*2^23: fp32 add/sub forces RNE-to-integer

# 'psum': ACT pre-fills the correction into the PSUM bank, matmuls
# accumulate onto it (start=False).  'sbuf': ACT writes the correction
# to SBUF and the combine adds it (2 DVE ops) with normal matmul groups.
PREFILL_MODE = "sbuf"

_PROGRAM = None
LAST_RESULTS = None


def _ensure_ntff_hook():
    """Install the axon NTFF profiling hook if the image's antenv lacks it.

    Best-effort: profiling only; compile/run work without it.
    """
    import contextlib
    import ctypes
    import types

    try:
        try:
            import antenv.axon_hooks as hooks_mod
        except ImportError:
            import antenv

            hooks_mod = types.ModuleType("antenv.axon_hooks")
            _holder = {}
            hooks_mod.set_axon_ntff_profile_hook = (
                lambda hook: _holder.__setitem__("hook", hook))
            hooks_mod.get_axon_ntff_profile_hook = (
                lambda: _holder.get("hook"))
            sys.modules["antenv.axon_hooks"] = hooks_mod
            antenv.axon_hooks = hooks_mod

        if hooks_mod.get_axon_ntff_profile_hook() is not None:
            return
        so_path = "/opt/axon/libaxon_pjrt.so"
        if not os.path.exists(so_path):
            return
        lib = ctypes.CDLL(so_path)
        if not hasattr(lib, "axon_start_nrt_profile"):
            return
        lib.axon_start_nrt_profile.argtypes = [
            ctypes.POINTER(ctypes.c_int64), ctypes.c_size_t]
        lib.axon_start_nrt_profile.restype = ctypes.c_int64
        lib.axon_stop_nrt_profile.argtypes = [ctypes.c_char_p]
        lib.axon_stop_nrt_profile.restype = ctypes.c_int64

        @contextlib.contextmanager
        def _hook(output_dir, device_ids):
            import jax

            jax.devices()
            if device_ids:
                ids = (ctypes.c_int64 * len(device_ids))(*device_ids)
                rc = lib.axon_start_nrt_profile(ids, len(device_ids))
            else:
                rc = lib.axon_start_nrt_profile(None, 0)
            if rc != 0:
                raise RuntimeError(f"axon_start_nrt_profile rc={rc}")
            try:
                yield
            finally:
                n = lib.axon_stop_nrt_profile(str(output_dir).encode())
                print(f"ntff profile: {n} file(s) written to {output_dir}")

        hooks_mod.set_axon_ntff_profile_hook(_hook)
    except Exception:
        pass


def _build_program():
    import concourse.mybir as mybir
    import concourse.tile as tile
    from concourse import bacc

    f16 = mybir.dt.float16
    f32 = mybir.dt.float32
    f8 = mybir.dt.float8e4
    Alu = mybir.AluOpType
    Act = mybir.ActivationFunctionType

    nc = bacc.Bacc(None, target_bir_lowering=False)

    # gathered x, token-major: cols 0..3839 int features, 3840..4095 fp
    xg_d = nc.dram_tensor("xg", [TPC, IN_F], f16, kind="ExternalInput")
    # merged weights, k-major fp8: wq[g,p,e,o] = Wm[e*128+p, g*512+o]
    # rows 0..3839 = raw int4 vals (fp8-exact), 3840..4095 = fp_w/wscale
    wq_d = nc.dram_tensor("wq", [NG, 128, KE, OG], f8, kind="ExternalInput")
    # wscale broadcast to 128 partitions
    wsb_d = nc.dram_tensor("wsb", [128, OUT_F], f16, kind="ExternalInput")
    # reduced_w/wscale broadcast to 128 partitions (corr prefill input)
    rwb_d = nc.dram_tensor("rwb", [128, OUT_F], f16, kind="ExternalInput")
    out_d = nc.dram_tensor("out", [TPC, OUT_F], f16, kind="ExternalOutput")
    debug = bool(os.environ.get("KBG_DEBUG"))
    if debug:
        dbg_q8 = nc.dram_tensor("dbg_q8", [2, 128, HKE, 128], f8,
                                kind="ExternalOutput")
        dbg_qa = nc.dram_tensor("dbg_qa", [2, 128, 2048], f16,
                                kind="ExternalOutput")
        dbg_pp = nc.dram_tensor("dbg_pp", [128, 4 * NT], f32,
                                kind="ExternalOutput")
        dbg_m = nc.dram_tensor("dbg_m", [128, OG], f32,
                               kind="ExternalOutput")

    with tile.TileContext(nc) as tc:
        with tc.tile_pool(name="consts", bufs=1) as consts, \
             tc.tile_pool(name="xin", bufs=3) as xin, \
             tc.tile_pool(name="y0p", bufs=2) as y0p, \
             tc.tile_pool(name="qap", bufs=2) as qap, \
             tc.tile_pool(name="qtp", bufs=2) as qtp, \
             tc.tile_pool(name="qt8", bufs=2 * NT) as qt8, \
             tc.tile_pool(name="wqp", bufs=4) as wqp, \
             tc.tile_pool(name="jnk", bufs=1) as jnk, \
             tc.tile_pool(name="stp", bufs=4) as stp, \
             tc.tile_pool(name="outp", bufs=4) as outp, \
             tc.tile_pool(name="corrp", bufs=4) as corrp, \
             tc.tile_pool(name="ps0", bufs=8, space="PSUM") as ps0:

            # --- sync HWDGE ring: x tiles only (transposes now run on
            # the PE, so this ring is otherwise empty)
            xts = []
            for r in range(NT):
                xt = xin.tile([128, IN_F], f16, tag="xt")
                nc.sync.dma_start(
                    out=xt[:, :], in_=xg_d[r * 128:(r + 1) * 128, :])
                xts.append(xt)

            # --- scalar HWDGE ring: weights only; first 4 groups queued
            # immediately so the ring never starves
            wq_tiles = []

            def load_wq(g, eng=None):
                wqg = wqp.tile([128, KE, OG], f8, name="wqg")
                eng = eng or nc.sync
                eng.dma_start(out=wqg[:, :, :], in_=wq_d[g, :, :, :])
                wq_tiles.append(wqg)

            # ALL weights ride the sync ring FIFO behind the x tiles: x
            # gets the full early HBM bandwidth, the ring never starves
            # (triggers all issue from the idle sync engine), and pool-slot
            # waits block nothing that matters
            load_wq(0)
            load_wq(1)

            # --- SWDGE: broadcast consts (needed only by ~30us; SWDGE
            # paces itself and stays off the x/weight ring)
            wsb_s = consts.tile([128, OUT_F], f16)
            nc.gpsimd.dma_start(out=wsb_s[:, :], in_=wsb_d[:, :])
            rwb_s = consts.tile([128, OUT_F], f16)
            nc.gpsimd.dma_start(out=rwb_s[:, :], in_=rwb_d[:, :])

            # [scale, rs, bq, alpha] packed per tile r at ppack[:, 4r:4r+4]
            ppack = consts.tile([128, 4 * NT], f32)

            def param(r, v):
                idx = 4 * r + v
                return ppack[:, idx:idx + 1]

            def stats_params(r):
                xt = xts[r]
                mn = stp.tile([128, 1], f32, tag="mn")
                mx = stp.tile([128, 1], f32, tag="mx")
                a1 = jnk.tile([128, 1920], f16, tag="a1")
                a2 = jnk.tile([128, 960], f16, tag="a2")
                nc.vector.tensor_tensor(
                    out=a1[:, :], in0=xt[:, :1920], in1=xt[:, 1920:INT_F],
                    op=Alu.min)
                nc.vector.tensor_tensor(
                    out=a2[:, :], in0=a1[:, :960], in1=a1[:, 960:], op=Alu.min)
                nc.vector.tensor_reduce(
                    out=mn[:, :], in_=a2[:, :], axis=mybir.AxisListType.X,
                    op=Alu.min)
                b1 = jnk.tile([128, 1920], f16, tag="a1")
                b2 = jnk.tile([128, 960], f16, tag="a2")
                nc.vector.tensor_tensor(
                    out=b1[:, :], in0=xt[:, :1920], in1=xt[:, 1920:INT_F],
                    op=Alu.max)
                nc.vector.tensor_tensor(
                    out=b2[:, :], in0=b1[:, :960], in1=b1[:, 960:], op=Alu.max)
                nc.vector.tensor_reduce(
                    out=mx[:, :], in_=b2[:, :], axis=mybir.AxisListType.X,
                    op=Alu.max)
                hp = tc.high_priority()
                hp.__enter__()
                d = stp.tile([128, 1], f32, tag="d")
                nc.vector.tensor_sub(d[:, :], mx[:, :], mn[:, :])
                nc.vector.tensor_scalar(
                    out=param(r, 0), in0=d[:, :],
                    scalar1=1.0 / 15.0, scalar2=1e-8, op0=Alu.mult,
                    op1=Alu.max)
                nc.vector.reciprocal(param(r, 1), param(r, 0))
                tt = stp.tile([128, 1], f32, tag="tt")
                nc.vector.tensor_mul(tt[:, :], mn[:, :], param(r, 1))
                # bq = -mn*rs - 8
                nc.vector.tensor_scalar(
                    out=param(r, 2), in0=tt[:, :],
                    scalar1=-1.0, scalar2=-8.0, op0=Alu.mult,
                    op1=Alu.add)
                # alpha = mn + 8*scale
                t8 = stp.tile([128, 1], f32, tag="t8")
                nc.vector.tensor_scalar(
                    out=t8[:, :], in0=param(r, 0),
                    scalar1=8.0, scalar2=None, op0=Alu.mult)
                nc.vector.tensor_add(param(r, 3), t8[:, :], mn[:, :])
                hp.__exit__(None, None, None)

            q8s = [[None, None] for _ in range(NT)]

            def phase_a(r):
                """stats -> quant+round per half -> xbar transpose (on the
                otherwise-empty scalar ring) -> DVE cast to fp8."""
                stats_params(r)
                xt = xts[r]
                qtbs = []
                for h in (0, 1):
                    c0 = h * 2048
                    ci = 2048 if h == 0 else INT_F - 2048
                    y0 = y0p.tile([128, 2048], f32, tag=f"y{h}")
                    nc.scalar.activation(
                        out=y0[:, :ci], in_=xt[:, c0:c0 + ci],
                        func=Act.Identity,
                        bias=param(r, 2), scale=param(r, 1))
                    qa = qap.tile([128, 2048], f16, tag=f"qa{h}")
                    # (y0+C)-C: fp32-internal RNE round to integer, f16 out
                    nc.vector.tensor_scalar(
                        out=qa[:, :ci], in0=y0[:, :ci], scalar1=C_MAGIC,
                        scalar2=-C_MAGIC, op0=Alu.add, op1=Alu.add)
                    if h == 1:
                        # fp cols: x_fp * rs (NOT rounded)
                        nc.scalar.activation(
                            out=qa[:, ci:], in_=xt[:, INT_F:],
                            func=Act.Identity, scale=param(r, 1))
                    qtb = qtp.tile([128, HKE, 128], f16, tag=f"qt{h}")
                    nc.scalar.dma_start_transpose(
                        out=qtb[:, :, :], in_=qa[:, :])
                    qtbs.append(qtb)
                for h in (0, 1):
                    q8 = qt8.tile([128, HKE, 128], f8, tag=f"q8_{r}{h}",
                                  bufs=1)
                    nc.vector.tensor_copy(
                        out=q8[:, :, :], in_=qtbs[h][:, :, :])
                    q8s[r][h] = q8

            def block(g, t):
                wqg = wq_tiles[g]
                o0 = g * OG
                t0 = t * 128
                # corr = alpha_t * (redw/wscale)_o on ACT; rides the
                # combine as the affine_then_add in1
                cr = corrp.tile([128, OG], f16, name="cr")
                nc.scalar.activation(
                    out=cr[:, :], in_=rwb_s[:, o0:o0 + OG],
                    func=Act.Identity, scale=param(t, 3))
                p0 = ps0.tile([128, OG], f32, name="p0")
                for e in range(KE // 2):
                    h, el = (0, e) if e < HKE // 2 else (1, e - HKE // 2)
                    nc.tensor.matmul(
                        p0[:, :], q8s[t][h][:, 2 * el:2 * el + 2, :],
                        wqg[:, 2 * e:2 * e + 2, :],
                        start=(e == 0), stop=(e == KE // 2 - 1),
                        perf_mode=mybir.MatmulPerfMode.DoubleRow)
                # out = (p0*scale_t + cr/wscale-fold)*wscale
                #     = scale_t*wscale*p0 + alpha*redw
                m = outp.tile([128, OG], f32, tag="m")
                nc.vector.affine_then_add(
                    out=m[:, :], in0=p0[:, :], in1=cr[:, :],
                    scale=param(t, 0), bias=0.0)
                ot = outp.tile([128, OG], f16, tag="ot")
                nc.vector.tensor_mul(ot[:, :], m[:, :], wsb_s[:, o0:o0 + OG])
                nc.gpsimd.dma_start(
                    out=out_d[t0:t0 + 128, o0:o0 + OG], in_=ot[:, :])

            # cohorts of 2/3/3 groups, t-outer inside a cohort: early HBM
            # demand stays low (x + 2 weight groups), and later weight
            # triggers are placed right after the block that frees the
            # buffer they rotate into
            for g in range(2, NG):
                load_wq(g)
            phase_a(0)
            block(0, 0)
            phase_a(1)
            block(1, 0)
            block(0, 1)
            phase_a(2)
            block(1, 1)
            block(0, 2)
            phase_a(3)
            block(1, 2)
            block(0, 3)
            block(1, 3)
            for t in range(NT):
                for gg in (2, 3, 4):
                    block(gg, t)
            for t in range(NT):
                for gg in (5, 6, 7):
                    block(gg, t)

    nc.finalize()
    return nc


def _get_program():
    global _PROGRAM
    if _PROGRAM is None:
        _PROGRAM = _build_program()
    return _PROGRAM


def _unpack_i4(w_packed):
    """(out, INT_F//2) uint8 -> (out, INT_F) int8; col 2k=low nibble, 2k+1=high."""
    lo = (w_packed & 0x0F).astype(np.int8)
    hi = ((w_packed >> 4) & 0x0F).astype(np.int8)
    lo = np.where(lo >= 8, lo - 16, lo)
    hi = np.where(hi >= 8, hi - 16, hi)
    w = np.empty((w_packed.shape[0], w_packed.shape[1] * 2), dtype=np.int8)
    w[:, 0::2] = lo
    w[:, 1::2] = hi
    return w


def _prep_inputs(x, int_weight, weights_scales, reduced_w, fp_weight, bias,
                 int_indices, fp_indices):
    import ml_dtypes
    f8np = ml_dtypes.float8_e4m3

    x2 = np.asarray(x, dtype=np.float16)[0]
    int_idx = np.asarray(int_indices).astype(np.int64)
    fp_idx = np.asarray(fp_indices).astype(np.int64)

    # gathered x: int cols then fp cols, token-major
    all_idx = np.concatenate([int_idx, fp_idx])
    xg = np.ascontiguousarray(x2[:, all_idx])               # (M, 4096) f16

    wsc = np.asarray(weights_scales).astype(np.float32)     # (OUT_F, 1)
    w_int = _unpack_i4(np.asarray(int_weight))              # (OUT_F, 3840) int8
    fpW = np.asarray(fp_weight).astype(np.float32)          # (OUT_F, 256)
    # merged k-major weights: rows 0..3839 raw int4, 3840..4095 fp_w/wsc
    Wm = np.empty((IN_F, OUT_F), dtype=np.float32)
    Wm[:INT_F, :] = w_int.T
    Wm[INT_F:, :] = (fpW / wsc).T
    wq = np.ascontiguousarray(
        Wm.reshape(KE, 128, NG, OG).transpose(2, 1, 0, 3)).astype(f8np)

    wsb = np.broadcast_to(
        wsc[:, 0].astype(np.float16)[None, :], (128, OUT_F)).copy()
    redw = np.asarray(reduced_w).astype(np.float32)         # (1, OUT_F)
    rwb = np.broadcast_to(
        (redw[0] / wsc[:, 0]).astype(np.float16)[None, :], (128, OUT_F)
    ).copy()

    in_maps = []
    for c in range(NCORES):
        tok = slice(c * TPC, (c + 1) * TPC)
        in_maps.append({"xg": xg[tok], "wq": wq, "wsb": wsb, "rwb": rwb})
    return in_maps


def kernel(x, int_weight, weights_scales, reduced_w, fp_weight, bias,
           int_indices, fp_indices):
    global LAST_RESULTS
    from concourse.bass_utils import run_bass_kernel_spmd

    _ensure_ntff_hook()
    in_maps = _prep_inputs(x, int_weight, weights_scales, reduced_w,
                           fp_weight, bias, int_indices, fp_indices)
    nc = _get_program()
    res = run_bass_kernel_spmd(nc, in_maps, core_ids=list(range(NCORES)))
    LAST_RESULTS = res
    out = np.concatenate([res.results[c]["out"] for c in range(NCORES)], axis=0)
    out = out[None].astype(np.float16)
    b = np.asarray(bias).astype(np.float32)
    if np.any(b):
        # bias is all-zero in this problem's setup; handled here for
        # completeness without spending a matmul K-row on it
        out = (out.astype(np.float32) + b[None, None, :]).astype(np.float16)
    return out


# revision 46
# speedup vs baseline: 1.1176x; 1.1176x over previous
"""MixedQLinear Trainium2 kernel — token-parallel, merged-K version.

Computation (per reference):
  x2 = x[0]                                  (M=4096, IN_F=4096) fp16
  int_x = x2[:, int_indices]                 (M, 3840)
  fp_x  = x2[:, fp_indices]                  (M, 256)
  per-token asym quant of int_x to int4:  scale=(mx-mn)/15, zero=mn
  q = round((int_x-zero)/scale) - 8          in [-8,7]
  out = scale*w_scale*(q @ w_int.T) + (zero+8*scale)*reduced_w + fp_x@fp_w.T + bias

Strategy: shard TOKENS across the 8 cores (512 each); every core holds
the full out_features dimension.  No collective is needed.

Key algebra: divide the fp weights by wscale on the host and the fp
activations by scale_t on the device, then the fp path rides INSIDE the
int matmul (K = 3840 int + 256 fp = 4096 exactly = 16 fp8 DoubleRow
matmuls), because the combine multiplies the whole psum by
scale_t*wscale_o.  The zero-point term (mn+8*scale)*reduced_w becomes
(mn*rs+8) * (reduced_w/wscale) * (scale*wscale) and is pre-filled into
the PSUM bank by the ACT engine before the matmuls accumulate onto it.
Final combine is ONE fused DVE op: out = (p0*scale_t)*wscale_bcast.

Phase A per 128-token tile (split in 2 K-chunks for latency):
  - min (DVE fused tensor_tensor_reduce) / max (Pool) stats,
  - params: scale, rs=1/scale, bqC=-mn*rs-8+C, alphas=mn*rs+8,
  - quant+round in ONE ACT op: y = x*rs + (bqC) in f32 (magic constant
    C=1.5*2^23 forces RNE-to-integer), DVE subtracts C casting to f16,
  - fp cols: ACT copy with scale=rs into the same qa staging,
  - DMA xbar transpose (sync ring) -> k-major, DVE cast f16->f8.

DMA queues: weights (16.8MB) alone on the scalar HWDGE ring; the 8
transposes alone on the sync HWDGE ring; x tiles, consts and output
stores on the gpsimd SWDGE queue.

Host side does only layout work: column gather, int4 unpack, weight
merge + fp8 cast, broadcasts, token slicing, concat of outputs.
"""

import os
import sys

import numpy as np

for _p in ("/opt/trn_rl_repo",):
    if _p not in sys.path and os.path.isdir(_p):
        sys.path.insert(0, _p)

TOKENS = 4096
IN_F = 4096
OUT_F = 4096
FP_F = 256
INT_F = IN_F - FP_F          # 3840
NCORES = 8
TPC = TOKENS // NCORES       # 512 tokens per core
NT = TPC // 128              # 4 token tiles per core
KE = IN_F // 128             # 32 k-planes (30 int + 2 fp)
HKE = KE // 2                # 16 planes per half-chunk
NG = 8                       # out-feature groups per core
OG = OUT_F // NG             # 512 out features per group
C_MAGIC = 12582912.0         # 1.5*2^23: fp32 add/sub forces RNE-to-integer

# 'psum': ACT pre-fills the correction into the PSUM bank, matmuls
# accumulate onto it (start=False).  'sbuf': ACT writes the correction
# to SBUF and the combine adds it (2 DVE ops) with normal matmul groups.
PREFILL_MODE = "sbuf"

_PROGRAM = None
LAST_RESULTS = None


def _ensure_ntff_hook():
    """Install the axon NTFF profiling hook if the image's antenv lacks it.

    Best-effort: profiling only; compile/run work without it.
    """
    import contextlib
    import ctypes
    import types

    try:
        try:
            import antenv.axon_hooks as hooks_mod
        except ImportError:
            import antenv

            hooks_mod = types.ModuleType("antenv.axon_hooks")
            _holder = {}
            hooks_mod.set_axon_ntff_profile_hook = (
                lambda hook: _holder.__setitem__("hook", hook))
            hooks_mod.get_axon_ntff_profile_hook = (
                lambda: _holder.get("hook"))
            sys.modules["antenv.axon_hooks"] = hooks_mod
            antenv.axon_hooks = hooks_mod

        if hooks_mod.get_axon_ntff_profile_hook() is not None:
            return
        so_path = "/opt/axon/libaxon_pjrt.so"
        if not os.path.exists(so_path):
            return
        lib = ctypes.CDLL(so_path)
        if not hasattr(lib, "axon_start_nrt_profile"):
            return
        lib.axon_start_nrt_profile.argtypes = [
            ctypes.POINTER(ctypes.c_int64), ctypes.c_size_t]
        lib.axon_start_nrt_profile.restype = ctypes.c_int64
        lib.axon_stop_nrt_profile.argtypes = [ctypes.c_char_p]
        lib.axon_stop_nrt_profile.restype = ctypes.c_int64

        @contextlib.contextmanager
        def _hook(output_dir, device_ids):
            import jax

            jax.devices()
            if device_ids:
                ids = (ctypes.c_int64 * len(device_ids))(*device_ids)
                rc = lib.axon_start_nrt_profile(ids, len(device_ids))
            else:
                rc = lib.axon_start_nrt_profile(None, 0)
            if rc != 0:
                raise RuntimeError(f"axon_start_nrt_profile rc={rc}")
            try:
                yield
            finally:
                n = lib.axon_stop_nrt_profile(str(output_dir).encode())
                print(f"ntff profile: {n} file(s) written to {output_dir}")

        hooks_mod.set_axon_ntff_profile_hook(_hook)
    except Exception:
        pass


def _build_program():
    import concourse.mybir as mybir
    import concourse.tile as tile
    from concourse import bacc

    f16 = mybir.dt.float16
    f32 = mybir.dt.float32
    f8 = mybir.dt.float8e4
    Alu = mybir.AluOpType
    Act = mybir.ActivationFunctionType

    nc = bacc.Bacc(None, target_bir_lowering=False)

    # gathered x, token-major: cols 0..3839 int features, 3840..4095 fp
    xg_d = nc.dram_tensor("xg", [TPC, IN_F], f16, kind="ExternalInput")
    # merged weights, k-major fp8: wq[g,p,e,o] = Wm[e*128+p, g*512+o]
    # rows 0..3839 = raw int4 vals (fp8-exact), 3840..4095 = fp_w/wscale
    wq_d = nc.dram_tensor("wq", [NG, 128, KE, OG], f8, kind="ExternalInput")
    # wscale broadcast to 128 partitions
    wsb_d = nc.dram_tensor("wsb", [128, OUT_F], f16, kind="ExternalInput")
    # reduced_w/wscale broadcast to 128 partitions (corr prefill input)
    rwb_d = nc.dram_tensor("rwb", [128, OUT_F], f16, kind="ExternalInput")
    # identity for PE transposes
    id_d = nc.dram_tensor("ident", [128, 128], f16, kind="ExternalInput")
    out_d = nc.dram_tensor("out", [TPC, OUT_F], f16, kind="ExternalOutput")
    debug = bool(os.environ.get("KBG_DEBUG"))
    if debug:
        dbg_q8 = nc.dram_tensor("dbg_q8", [2, 128, HKE, 128], f8,
                                kind="ExternalOutput")
        dbg_qa = nc.dram_tensor("dbg_qa", [2, 128, 2048], f16,
                                kind="ExternalOutput")
        dbg_pp = nc.dram_tensor("dbg_pp", [128, 4 * NT], f32,
                                kind="ExternalOutput")
        dbg_m = nc.dram_tensor("dbg_m", [128, OG], f32,
                               kind="ExternalOutput")

    with tile.TileContext(nc) as tc:
        with tc.tile_pool(name="consts", bufs=1) as consts, \
             tc.tile_pool(name="xin", bufs=3) as xin, \
             tc.tile_pool(name="y0p", bufs=2) as y0p, \
             tc.tile_pool(name="qap", bufs=2) as qap, \
             tc.tile_pool(name="qt8", bufs=2 * NT) as qt8, \
             tc.tile_pool(name="wqp", bufs=5) as wqp, \
             tc.tile_pool(name="jnk", bufs=1) as jnk, \
             tc.tile_pool(name="stp", bufs=4) as stp, \
             tc.tile_pool(name="outp", bufs=4) as outp, \
             tc.tile_pool(name="corrp", bufs=4) as corrp, \
             tc.tile_pool(name="ps0", bufs=6, space="PSUM") as ps0, \
             tc.tile_pool(name="pstr", bufs=1, space="PSUM") as pstr:

            # --- sync HWDGE ring: x tiles only (transposes now run on
            # the PE, so this ring is otherwise empty)
            xts = []
            for r in range(NT):
                xt = xin.tile([128, IN_F], f16, tag="xt")
                nc.sync.dma_start(
                    out=xt[:, :], in_=xg_d[r * 128:(r + 1) * 128, :])
                xts.append(xt)

            # --- scalar HWDGE ring: weights only; first 4 groups queued
            # immediately so the ring never starves
            wq_tiles = []

            def load_wq(g, eng=None):
                wqg = wqp.tile([128, KE, OG], f8, name="wqg")
                eng = eng or nc.sync
                eng.dma_start(out=wqg[:, :, :], in_=wq_d[g, :, :, :])
                wq_tiles.append(wqg)

            # ALL weights ride the sync ring FIFO behind the x tiles: x
            # gets the full early HBM bandwidth, the ring never starves
            # (triggers all issue from the idle sync engine), and pool-slot
            # waits block nothing that matters
            load_wq(0)
            load_wq(1)

            # --- SWDGE: identity + broadcast consts (stores join later)
            id_s = consts.tile([128, 128], f16)
            nc.gpsimd.dma_start(out=id_s[:, :], in_=id_d[:, :])
            wsb_s = consts.tile([128, OUT_F], f16)
            nc.gpsimd.dma_start(out=wsb_s[:, :], in_=wsb_d[:, :])
            rwb_s = consts.tile([128, OUT_F], f16)
            nc.gpsimd.dma_start(out=rwb_s[:, :], in_=rwb_d[:, :])

            # [scale, rs, bq, alpha] packed per tile r at ppack[:, 4r:4r+4]
            ppack = consts.tile([128, 4 * NT], f32)

            def param(r, v):
                idx = 4 * r + v
                return ppack[:, idx:idx + 1]

            def stats_params(r):
                xt = xts[r]
                mn = stp.tile([128, 1], f32, tag="mn")
                mx = stp.tile([128, 1], f32, tag="mx")
                a1 = jnk.tile([128, 1920], f16, tag="a1")
                a2 = jnk.tile([128, 960], f16, tag="a2")
                nc.vector.tensor_tensor(
                    out=a1[:, :], in0=xt[:, :1920], in1=xt[:, 1920:INT_F],
                    op=Alu.min)
                nc.vector.tensor_tensor(
                    out=a2[:, :], in0=a1[:, :960], in1=a1[:, 960:], op=Alu.min)
                nc.vector.tensor_reduce(
                    out=mn[:, :], in_=a2[:, :], axis=mybir.AxisListType.X,
                    op=Alu.min)
                b1 = jnk.tile([128, 1920], f16, tag="a1")
                b2 = jnk.tile([128, 960], f16, tag="a2")
                nc.vector.tensor_tensor(
                    out=b1[:, :], in0=xt[:, :1920], in1=xt[:, 1920:INT_F],
                    op=Alu.max)
                nc.vector.tensor_tensor(
                    out=b2[:, :], in0=b1[:, :960], in1=b1[:, 960:], op=Alu.max)
                nc.vector.tensor_reduce(
                    out=mx[:, :], in_=b2[:, :], axis=mybir.AxisListType.X,
                    op=Alu.max)
                hp = tc.high_priority()
                hp.__enter__()
                d = stp.tile([128, 1], f32, tag="d")
                nc.vector.tensor_sub(d[:, :], mx[:, :], mn[:, :])
                nc.vector.tensor_scalar(
                    out=param(r, 0), in0=d[:, :],
                    scalar1=1.0 / 15.0, scalar2=1e-8, op0=Alu.mult,
                    op1=Alu.max)
                nc.vector.reciprocal(param(r, 1), param(r, 0))
                tt = stp.tile([128, 1], f32, tag="tt")
                nc.vector.tensor_mul(tt[:, :], mn[:, :], param(r, 1))
                # bq = -mn*rs - 8
                nc.vector.tensor_scalar(
                    out=param(r, 2), in0=tt[:, :],
                    scalar1=-1.0, scalar2=-8.0, op0=Alu.mult,
                    op1=Alu.add)
                # alpha = mn + 8*scale
                t8 = stp.tile([128, 1], f32, tag="t8")
                nc.vector.tensor_scalar(
                    out=t8[:, :], in0=param(r, 0),
                    scalar1=8.0, scalar2=None, op0=Alu.mult)
                nc.vector.tensor_add(param(r, 3), t8[:, :], mn[:, :])
                hp.__exit__(None, None, None)

            q8s = [[None, None] for _ in range(NT)]

            def phase_a(r):
                """stats -> per 1024-col quarter: quant (ACT), round (DVE),
                PE transpose (8x128), DVE cast to fp8.  Quarter granularity
                keeps every engine's piece short so the k-major planes
                trickle out with minimal latency."""
                stats_params(r)
                xt = xts[r]
                for h in (0, 1):
                    q8 = qt8.tile([128, HKE, 128], f8, tag=f"q8_{r}{h}",
                                  bufs=1)
                    q8s[r][h] = q8
                for q in range(4):
                    c0 = q * 1024
                    ci = 1024 if q < 3 else INT_F - 3072   # int cols (768)
                    y0 = y0p.tile([128, 1024], f32, tag=f"y{q % 2}")
                    nc.scalar.activation(
                        out=y0[:, :ci], in_=xt[:, c0:c0 + ci],
                        func=Act.Identity,
                        bias=param(r, 2), scale=param(r, 1))
                    qa = qap.tile([128, 1024], f16, tag=f"qa{q % 2}")
                    # (y0+C)-C: fp32-internal RNE round to integer, f16 out
                    nc.vector.tensor_scalar(
                        out=qa[:, :ci], in0=y0[:, :ci], scalar1=C_MAGIC,
                        scalar2=-C_MAGIC, op0=Alu.add, op1=Alu.add)
                    if q == 3:
                        # fp cols: x_fp * rs (NOT rounded)
                        nc.scalar.activation(
                            out=qa[:, ci:], in_=xt[:, INT_F:],
                            func=Act.Identity, scale=param(r, 1))
                    pt = pstr.tile([128, 8, 128], f16, tag=f"pt{q % 2}")
                    for j in range(8):
                        nc.tensor.transpose(
                            pt[:, j, :], qa[:, j * 128:(j + 1) * 128],
                            id_s[:, :])
                    h, c = q // 2, q % 2
                    nc.vector.tensor_copy(
                        out=q8s[r][h][:, c * 8:(c + 1) * 8, :],
                        in_=pt[:, :, :])

            def block(g, t):
                wqg = wq_tiles[g]
                o0 = g * OG
                t0 = t * 128
                # corr = alpha_t * (redw/wscale)_o on ACT; rides the
                # combine as the affine_then_add in1
                cr = corrp.tile([128, OG], f16, name="cr")
                nc.scalar.activation(
                    out=cr[:, :], in_=rwb_s[:, o0:o0 + OG],
                    func=Act.Identity, scale=param(t, 3))
                p0 = ps0.tile([128, OG], f32, name="p0")
                for e in range(KE // 2):
                    h, el = (0, e) if e < HKE // 2 else (1, e - HKE // 2)
                    nc.tensor.matmul(
                        p0[:, :], q8s[t][h][:, 2 * el:2 * el + 2, :],
                        wqg[:, 2 * e:2 * e + 2, :],
                        start=(e == 0), stop=(e == KE // 2 - 1),
                        perf_mode=mybir.MatmulPerfMode.DoubleRow)
                # out = (p0*scale_t + cr/wscale-fold)*wscale
                #     = scale_t*wscale*p0 + alpha*redw
                m = outp.tile([128, OG], f32, tag="m")
                nc.vector.affine_then_add(
                    out=m[:, :], in0=p0[:, :], in1=cr[:, :],
                    scale=param(t, 0), bias=0.0)
                ot = outp.tile([128, OG], f16, tag="ot")
                nc.vector.tensor_mul(ot[:, :], m[:, :], wsb_s[:, o0:o0 + OG])
                nc.gpsimd.dma_start(
                    out=out_d[t0:t0 + 128, o0:o0 + OG], in_=ot[:, :])

            # cohorts of 2/3/3 groups, t-outer inside a cohort: early HBM
            # demand stays low (x + 2 weight groups), and later weight
            # triggers are placed right after the block that frees the
            # buffer they rotate into
            for g in range(2, NG):
                load_wq(g)
            phase_a(0)
            block(0, 0)
            phase_a(1)
            block(1, 0)
            block(0, 1)
            phase_a(2)
            block(1, 1)
            block(0, 2)
            phase_a(3)
            block(1, 2)
            block(0, 3)
            block(1, 3)
            for t in range(NT):
                for gg in (2, 3, 4):
                    block(gg, t)
            for t in range(NT):
                for gg in (5, 6, 7):
                    block(gg, t)

    nc.finalize()
    return nc


def _get_program():
    global _PROGRAM
    if _PROGRAM is None:
        _PROGRAM = _build_program()
    return _PROGRAM


def _unpack_i4(w_packed):
    """(out, INT_F//2) uint8 -> (out, INT_F) int8; col 2k=low nibble, 2k+1=high."""
    lo = (w_packed & 0x0F).astype(np.int8)
    hi = ((w_packed >> 4) & 0x0F).astype(np.int8)
    lo = np.where(lo >= 8, lo - 16, lo)
    hi = np.where(hi >= 8, hi - 16, hi)
    w = np.empty((w_packed.shape[0], w_packed.shape[1] * 2), dtype=np.int8)
    w[:, 0::2] = lo
    w[:, 1::2] = hi
    return w


def _prep_inputs(x, int_weight, weights_scales, reduced_w, fp_weight, bias,
                 int_indices, fp_indices):
    import ml_dtypes
    f8np = ml_dtypes.float8_e4m3

    x2 = np.asarray(x, dtype=np.float16)[0]
    int_idx = np.asarray(int_indices).astype(np.int64)
    fp_idx = np.asarray(fp_indices).astype(np.int64)

    # gathered x: int cols then fp cols, token-major
    all_idx = np.concatenate([int_idx, fp_idx])
    xg = np.ascontiguousarray(x2[:, all_idx])               # (M, 4096) f16

    wsc = np.asarray(weights_scales).astype(np.float32)     # (OUT_F, 1)
    w_int = _unpack_i4(np.asarray(int_weight))              # (OUT_F, 3840) int8
    fpW = np.asarray(fp_weight).astype(np.float32)          # (OUT_F, 256)
    # merged k-major weights: rows 0..3839 raw int4, 3840..4095 fp_w/wsc
    Wm = np.empty((IN_F, OUT_F), dtype=np.float32)
    Wm[:INT_F, :] = w_int.T
    Wm[INT_F:, :] = (fpW / wsc).T
    wq = np.ascontiguousarray(
        Wm.reshape(KE, 128, NG, OG).transpose(2, 1, 0, 3)).astype(f8np)

    wsb = np.broadcast_to(
        wsc[:, 0].astype(np.float16)[None, :], (128, OUT_F)).copy()
    redw = np.asarray(reduced_w).astype(np.float32)         # (1, OUT_F)
    rwb = np.broadcast_to(
        (redw[0] / wsc[:, 0]).astype(np.float16)[None, :], (128, OUT_F)
    ).copy()

    ident = np.eye(128, dtype=np.float16)
    in_maps = []
    for c in range(NCORES):
        tok = slice(c * TPC, (c + 1) * TPC)
        in_maps.append({"xg": xg[tok], "wq": wq, "wsb": wsb, "rwb": rwb,
                        "ident": ident})
    return in_maps


def kernel(x, int_weight, weights_scales, reduced_w, fp_weight, bias,
           int_indices, fp_indices):
    global LAST_RESULTS
    from concourse.bass_utils import run_bass_kernel_spmd

    _ensure_ntff_hook()
    in_maps = _prep_inputs(x, int_weight, weights_scales, reduced_w,
                           fp_weight, bias, int_indices, fp_indices)
    nc = _get_program()
    res = run_bass_kernel_spmd(nc, in_maps, core_ids=list(range(NCORES)))
    LAST_RESULTS = res
    out = np.concatenate([res.results[c]["out"] for c in range(NCORES)], axis=0)
    out = out[None].astype(np.float16)
    b = np.asarray(bias).astype(np.float32)
    if np.any(b):
        # bias is all-zero in this problem's setup; handled here for
        # completeness without spending a matmul K-row on it
        out = (out.astype(np.float32) + b[None, None, :]).astype(np.float16)
    return out


# revision 47
# speedup vs baseline: 1.1954x; 1.0696x over previous
"""MixedQLinear Trainium2 kernel — token-parallel, merged-K version.

Computation (per reference):
  x2 = x[0]                                  (M=4096, IN_F=4096) fp16
  int_x = x2[:, int_indices]                 (M, 3840)
  fp_x  = x2[:, fp_indices]                  (M, 256)
  per-token asym quant of int_x to int4:  scale=(mx-mn)/15, zero=mn
  q = round((int_x-zero)/scale) - 8          in [-8,7]
  out = scale*w_scale*(q @ w_int.T) + (zero+8*scale)*reduced_w + fp_x@fp_w.T + bias

Strategy: shard TOKENS across the 8 cores (512 each); every core holds
the full out_features dimension.  No collective is needed.

Key algebra: divide the fp weights by wscale on the host and the fp
activations by scale_t on the device, then the fp path rides INSIDE the
int matmul (K = 3840 int + 256 fp = 4096 exactly = 16 fp8 DoubleRow
matmuls), because the combine multiplies the whole psum by
scale_t*wscale_o.  The zero-point term (mn+8*scale)*reduced_w becomes
(mn*rs+8) * (reduced_w/wscale) * (scale*wscale) and is pre-filled into
the PSUM bank by the ACT engine before the matmuls accumulate onto it.
Final combine is ONE fused DVE op: out = (p0*scale_t)*wscale_bcast.

Phase A per 128-token tile (split in 2 K-chunks for latency):
  - min (DVE fused tensor_tensor_reduce) / max (Pool) stats,
  - params: scale, rs=1/scale, bqC=-mn*rs-8+C, alphas=mn*rs+8,
  - quant+round in ONE ACT op: y = x*rs + (bqC) in f32 (magic constant
    C=1.5*2^23 forces RNE-to-integer), DVE subtracts C casting to f16,
  - fp cols: ACT copy with scale=rs into the same qa staging,
  - DMA xbar transpose (sync ring) -> k-major, DVE cast f16->f8.

DMA queues: weights (16.8MB) alone on the scalar HWDGE ring; the 8
transposes alone on the sync HWDGE ring; x tiles, consts and output
stores on the gpsimd SWDGE queue.

Host side does only layout work: column gather, int4 unpack, weight
merge + fp8 cast, broadcasts, token slicing, concat of outputs.
"""

import os
import sys

import numpy as np

for _p in ("/opt/trn_rl_repo",):
    if _p not in sys.path and os.path.isdir(_p):
        sys.path.insert(0, _p)

TOKENS = 4096
IN_F = 4096
OUT_F = 4096
FP_F = 256
INT_F = IN_F - FP_F          # 3840
NCORES = 8
TPC = TOKENS // NCORES       # 512 tokens per core
NT = TPC // 128              # 4 token tiles per core
KE = IN_F // 128             # 32 k-planes (30 int + 2 fp)
HKE = KE // 2                # 16 planes per half-chunk
NG = 8                       # out-feature groups per core
OG = OUT_F // NG             # 512 out features per group
C_MAGIC = 12582912.0         # 1.5*2^23: fp32 add/sub forces RNE-to-integer

# 'psum': ACT pre-fills the correction into the PSUM bank, matmuls
# accumulate onto it (start=False).  'sbuf': ACT writes the correction
# to SBUF and the combine adds it (2 DVE ops) with normal matmul groups.
PREFILL_MODE = "sbuf"

_PROGRAM = None
LAST_RESULTS = None


def _ensure_ntff_hook():
    """Install the axon NTFF profiling hook if the image's antenv lacks it.

    Best-effort: profiling only; compile/run work without it.
    """
    import contextlib
    import ctypes
    import types

    try:
        try:
            import antenv.axon_hooks as hooks_mod
        except ImportError:
            import antenv

            hooks_mod = types.ModuleType("antenv.axon_hooks")
            _holder = {}
            hooks_mod.set_axon_ntff_profile_hook = (
                lambda hook: _holder.__setitem__("hook", hook))
            hooks_mod.get_axon_ntff_profile_hook = (
                lambda: _holder.get("hook"))
            sys.modules["antenv.axon_hooks"] = hooks_mod
            antenv.axon_hooks = hooks_mod

        if hooks_mod.get_axon_ntff_profile_hook() is not None:
            return
        so_path = "/opt/axon/libaxon_pjrt.so"
        if not os.path.exists(so_path):
            return
        lib = ctypes.CDLL(so_path)
        if not hasattr(lib, "axon_start_nrt_profile"):
            return
        lib.axon_start_nrt_profile.argtypes = [
            ctypes.POINTER(ctypes.c_int64), ctypes.c_size_t]
        lib.axon_start_nrt_profile.restype = ctypes.c_int64
        lib.axon_stop_nrt_profile.argtypes = [ctypes.c_char_p]
        lib.axon_stop_nrt_profile.restype = ctypes.c_int64

        @contextlib.contextmanager
        def _hook(output_dir, device_ids):
            import jax

            jax.devices()
            if device_ids:
                ids = (ctypes.c_int64 * len(device_ids))(*device_ids)
                rc = lib.axon_start_nrt_profile(ids, len(device_ids))
            else:
                rc = lib.axon_start_nrt_profile(None, 0)
            if rc != 0:
                raise RuntimeError(f"axon_start_nrt_profile rc={rc}")
            try:
                yield
            finally:
                n = lib.axon_stop_nrt_profile(str(output_dir).encode())
                print(f"ntff profile: {n} file(s) written to {output_dir}")

        hooks_mod.set_axon_ntff_profile_hook(_hook)
    except Exception:
        pass


def _build_program():
    import concourse.mybir as mybir
    import concourse.tile as tile
    from concourse import bacc

    f16 = mybir.dt.float16
    f32 = mybir.dt.float32
    f8 = mybir.dt.float8e4
    Alu = mybir.AluOpType
    Act = mybir.ActivationFunctionType

    nc = bacc.Bacc(None, target_bir_lowering=False)

    # gathered x, token-major: cols 0..3839 int features, 3840..4095 fp
    xg_d = nc.dram_tensor("xg", [TPC, IN_F], f16, kind="ExternalInput")
    # merged weights, k-major fp8: wq[g,p,e,o] = Wm[e*128+p, g*512+o]
    # rows 0..3839 = raw int4 vals (fp8-exact), 3840..4095 = fp_w/wscale
    wq_d = nc.dram_tensor("wq", [NG, 128, KE, OG], f8, kind="ExternalInput")
    # wscale broadcast to 128 partitions
    wsb_d = nc.dram_tensor("wsb", [128, OUT_F], f16, kind="ExternalInput")
    # reduced_w/wscale broadcast to 128 partitions (corr prefill input)
    rwb_d = nc.dram_tensor("rwb", [128, OUT_F], f16, kind="ExternalInput")
    # identity for PE transposes
    id_d = nc.dram_tensor("ident", [128, 128], f16, kind="ExternalInput")
    out_d = nc.dram_tensor("out", [TPC, OUT_F], f16, kind="ExternalOutput")
    debug = bool(os.environ.get("KBG_DEBUG"))
    if debug:
        dbg_q8 = nc.dram_tensor("dbg_q8", [2, 128, HKE, 128], f8,
                                kind="ExternalOutput")
        dbg_qa = nc.dram_tensor("dbg_qa", [2, 128, 2048], f16,
                                kind="ExternalOutput")
        dbg_pp = nc.dram_tensor("dbg_pp", [128, 4 * NT], f32,
                                kind="ExternalOutput")
        dbg_m = nc.dram_tensor("dbg_m", [128, OG], f32,
                               kind="ExternalOutput")

    with tile.TileContext(nc) as tc:
        with tc.tile_pool(name="consts", bufs=1) as consts, \
             tc.tile_pool(name="xin", bufs=3) as xin, \
             tc.tile_pool(name="y0p", bufs=2) as y0p, \
             tc.tile_pool(name="qap", bufs=2) as qap, \
             tc.tile_pool(name="qt8", bufs=2 * NT) as qt8, \
             tc.tile_pool(name="wqp", bufs=5) as wqp, \
             tc.tile_pool(name="jnk", bufs=1) as jnk, \
             tc.tile_pool(name="stp", bufs=4) as stp, \
             tc.tile_pool(name="outp", bufs=4) as outp, \
             tc.tile_pool(name="corrp", bufs=4) as corrp, \
             tc.tile_pool(name="ps0", bufs=5, space="PSUM") as ps0, \
             tc.tile_pool(name="pstr", bufs=3, space="PSUM") as pstr:

            # --- sync HWDGE ring: x tiles (in half-K chunks so stats can
            # start on the first half while the second streams)
            xts = []
            for r in range(NT):
                xa = xin.tile([128, 2048], f16, tag="xa")
                nc.sync.dma_start(
                    out=xa[:, :], in_=xg_d[r * 128:(r + 1) * 128, :2048])
                xb = xin.tile([128, 2048], f16, tag="xb")
                nc.sync.dma_start(
                    out=xb[:, :], in_=xg_d[r * 128:(r + 1) * 128, 2048:])
                xts.append((xa, xb))

            # --- scalar HWDGE ring: weights only; first 4 groups queued
            # immediately so the ring never starves
            wq_tiles = []

            def load_wq(g, eng=None):
                wqg = wqp.tile([128, KE, OG], f8, name="wqg")
                eng = eng or nc.sync
                eng.dma_start(out=wqg[:, :, :], in_=wq_d[g, :, :, :])
                wq_tiles.append(wqg)

            # ALL weights ride the sync ring FIFO behind the x tiles: x
            # gets the full early HBM bandwidth, the ring never starves
            # (triggers all issue from the idle sync engine), and pool-slot
            # waits block nothing that matters
            load_wq(0)
            load_wq(1)

            # --- SWDGE: identity + broadcast consts (stores join later)
            id_s = consts.tile([128, 128], f16)
            nc.gpsimd.dma_start(out=id_s[:, :], in_=id_d[:, :])
            wsb_s = consts.tile([128, OUT_F], f16)
            nc.gpsimd.dma_start(out=wsb_s[:, :], in_=wsb_d[:, :])
            rwb_s = consts.tile([128, OUT_F], f16)
            nc.gpsimd.dma_start(out=rwb_s[:, :], in_=rwb_d[:, :])

            # [scale, rs, bq, alpha] packed per tile r at ppack[:, 4r:4r+4]
            ppack = consts.tile([128, 4 * NT], f32)

            def param(r, v):
                idx = 4 * r + v
                return ppack[:, idx:idx + 1]

            def stats_params(r):
                xa, xb = xts[r]
                mn = stp.tile([128, 1], f32, tag="mn")
                mx = stp.tile([128, 1], f32, tag="mx")
                mnx = [None] * 4
                # per-half min/max trees: half A runs while half B streams
                for hi, (xh, ilen) in enumerate(((xa, 2048), (xb, 1792))):
                    l1 = ilen // 2
                    for oi, op in enumerate((Alu.min, Alu.max)):
                        a1 = jnk.tile([128, 1024], f16, tag="a1")
                        a2 = jnk.tile([128, 512], f16, tag="a2")
                        pp = stp.tile([128, 1], f32, tag=f"pp{hi}{oi}")
                        nc.vector.tensor_tensor(
                            out=a1[:, :l1], in0=xh[:, :l1],
                            in1=xh[:, l1:ilen], op=op)
                        nc.vector.tensor_tensor(
                            out=a2[:, :l1 // 2], in0=a1[:, :l1 // 2],
                            in1=a1[:, l1 // 2:l1], op=op)
                        nc.vector.tensor_reduce(
                            out=pp[:, :], in_=a2[:, :l1 // 2],
                            axis=mybir.AxisListType.X, op=op)
                        mnx[2 * hi + oi] = pp
                nc.vector.tensor_tensor(
                    out=mn[:, :], in0=mnx[0][:, :], in1=mnx[2][:, :],
                    op=Alu.min)
                nc.vector.tensor_tensor(
                    out=mx[:, :], in0=mnx[1][:, :], in1=mnx[3][:, :],
                    op=Alu.max)
                hp = tc.high_priority()
                hp.__enter__()
                d = stp.tile([128, 1], f32, tag="d")
                nc.vector.tensor_sub(d[:, :], mx[:, :], mn[:, :])
                nc.vector.tensor_scalar(
                    out=param(r, 0), in0=d[:, :],
                    scalar1=1.0 / 15.0, scalar2=1e-8, op0=Alu.mult,
                    op1=Alu.max)
                nc.vector.reciprocal(param(r, 1), param(r, 0))
                tt = stp.tile([128, 1], f32, tag="tt")
                nc.vector.tensor_mul(tt[:, :], mn[:, :], param(r, 1))
                # bq = -mn*rs - 8
                nc.vector.tensor_scalar(
                    out=param(r, 2), in0=tt[:, :],
                    scalar1=-1.0, scalar2=-8.0, op0=Alu.mult,
                    op1=Alu.add)
                # alpha = mn + 8*scale
                t8 = stp.tile([128, 1], f32, tag="t8")
                nc.vector.tensor_scalar(
                    out=t8[:, :], in0=param(r, 0),
                    scalar1=8.0, scalar2=None, op0=Alu.mult)
                nc.vector.tensor_add(param(r, 3), t8[:, :], mn[:, :])
                hp.__exit__(None, None, None)

            q8s = [[None, None] for _ in range(NT)]

            def phase_a(r):
                """stats -> quant+round per half -> PE transpose -> cast."""
                stats_params(r)
                for h in (0, 1):
                    q8 = qt8.tile([128, HKE, 128], f8, tag=f"q8_{r}{h}",
                                  bufs=1)
                    q8s[r][h] = q8
                for h in (0, 1):
                    xh = xts[r][h]
                    ci = 2048 if h == 0 else INT_F - 2048
                    y0 = y0p.tile([128, 2048], f32, tag=f"y{h}")
                    nc.scalar.activation(
                        out=y0[:, :ci], in_=xh[:, :ci],
                        func=Act.Identity,
                        bias=param(r, 2), scale=param(r, 1))
                    qa = qap.tile([128, 2048], f16, tag=f"qa{h}")
                    # (y0+C)-C: fp32-internal RNE round to integer, f16 out
                    nc.vector.tensor_scalar(
                        out=qa[:, :ci], in0=y0[:, :ci], scalar1=C_MAGIC,
                        scalar2=-C_MAGIC, op0=Alu.add, op1=Alu.add)
                    if h == 1:
                        # fp cols: x_fp * rs (NOT rounded)
                        nc.scalar.activation(
                            out=qa[:, ci:], in_=xh[:, ci:],
                            func=Act.Identity, scale=param(r, 1))
                    # k-major via PE transpose, 2 chunks of 8 planes;
                    # ACT casts each psum chunk to fp8 in SBUF
                    for c in (0, 1):
                        pt = pstr.tile([128, 8, 128], f16, tag="pt")
                        for j in range(8):
                            col = c * 1024 + j * 128
                            nc.tensor.transpose(
                                pt[:, j, :], qa[:, col:col + 128],
                                id_s[:, :])
                        nc.scalar.copy(
                            q8s[r][h][:, c * 8:(c + 1) * 8, :], pt[:, :, :])

            def block(g, t):
                wqg = wq_tiles[g]
                o0 = g * OG
                t0 = t * 128
                # corr = alpha_t * (redw/wscale)_o on ACT; rides the
                # combine as the affine_then_add in1
                cr = corrp.tile([128, OG], f16, name="cr")
                nc.scalar.activation(
                    out=cr[:, :], in_=rwb_s[:, o0:o0 + OG],
                    func=Act.Identity, scale=param(t, 3))
                p0 = ps0.tile([128, OG], f32, name="p0")
                for e in range(KE // 2):
                    h, el = (0, e) if e < HKE // 2 else (1, e - HKE // 2)
                    nc.tensor.matmul(
                        p0[:, :], q8s[t][h][:, 2 * el:2 * el + 2, :],
                        wqg[:, 2 * e:2 * e + 2, :],
                        start=(e == 0), stop=(e == KE // 2 - 1),
                        perf_mode=mybir.MatmulPerfMode.DoubleRow)
                # out = (p0*scale_t + cr/wscale-fold)*wscale
                #     = scale_t*wscale*p0 + alpha*redw
                m = outp.tile([128, OG], f32, tag="m")
                nc.vector.affine_then_add(
                    out=m[:, :], in0=p0[:, :], in1=cr[:, :],
                    scale=param(t, 0), bias=0.0)
                ot = outp.tile([128, OG], f16, tag="ot")
                nc.vector.tensor_mul(ot[:, :], m[:, :], wsb_s[:, o0:o0 + OG])
                nc.gpsimd.dma_start(
                    out=out_d[t0:t0 + 128, o0:o0 + OG], in_=ot[:, :])

            # cohorts of 2/3/3 groups, t-outer inside a cohort: early HBM
            # demand stays low (x + 2 weight groups), and later weight
            # triggers are placed right after the block that frees the
            # buffer they rotate into
            for g in range(2, NG):
                load_wq(g)
            phase_a(0)
            block(0, 0)
            phase_a(1)
            block(1, 0)
            block(0, 1)
            phase_a(2)
            block(1, 1)
            block(0, 2)
            phase_a(3)
            block(1, 2)
            block(0, 3)
            block(1, 3)
            for t in range(NT):
                for gg in (2, 3, 4):
                    block(gg, t)
            for t in range(NT):
                for gg in (5, 6, 7):
                    block(gg, t)

    nc.finalize()
    return nc


def _get_program():
    global _PROGRAM
    if _PROGRAM is None:
        _PROGRAM = _build_program()
    return _PROGRAM


def _unpack_i4(w_packed):
    """(out, INT_F//2) uint8 -> (out, INT_F) int8; col 2k=low nibble, 2k+1=high."""
    lo = (w_packed & 0x0F).astype(np.int8)
    hi = ((w_packed >> 4) & 0x0F).astype(np.int8)
    lo = np.where(lo >= 8, lo - 16, lo)
    hi = np.where(hi >= 8, hi - 16, hi)
    w = np.empty((w_packed.shape[0], w_packed.shape[1] * 2), dtype=np.int8)
    w[:, 0::2] = lo
    w[:, 1::2] = hi
    return w


def _prep_inputs(x, int_weight, weights_scales, reduced_w, fp_weight, bias,
                 int_indices, fp_indices):
    import ml_dtypes
    f8np = ml_dtypes.float8_e4m3

    x2 = np.asarray(x, dtype=np.float16)[0]
    int_idx = np.asarray(int_indices).astype(np.int64)
    fp_idx = np.asarray(fp_indices).astype(np.int64)

    # gathered x: int cols then fp cols, token-major
    all_idx = np.concatenate([int_idx, fp_idx])
    xg = np.ascontiguousarray(x2[:, all_idx])               # (M, 4096) f16

    wsc = np.asarray(weights_scales).astype(np.float32)     # (OUT_F, 1)
    w_int = _unpack_i4(np.asarray(int_weight))              # (OUT_F, 3840) int8
    fpW = np.asarray(fp_weight).astype(np.float32)          # (OUT_F, 256)
    # merged k-major weights: rows 0..3839 raw int4, 3840..4095 fp_w/wsc
    Wm = np.empty((IN_F, OUT_F), dtype=np.float32)
    Wm[:INT_F, :] = w_int.T
    Wm[INT_F:, :] = (fpW / wsc).T
    wq = np.ascontiguousarray(
        Wm.reshape(KE, 128, NG, OG).transpose(2, 1, 0, 3)).astype(f8np)

    wsb = np.broadcast_to(
        wsc[:, 0].astype(np.float16)[None, :], (128, OUT_F)).copy()
    redw = np.asarray(reduced_w).astype(np.float32)         # (1, OUT_F)
    rwb = np.broadcast_to(
        (redw[0] / wsc[:, 0]).astype(np.float16)[None, :], (128, OUT_F)
    ).copy()

    ident = np.eye(128, dtype=np.float16)
    in_maps = []
    for c in range(NCORES):
        tok = slice(c * TPC, (c + 1) * TPC)
        in_maps.append({"xg": xg[tok], "wq": wq, "wsb": wsb, "rwb": rwb,
                        "ident": ident})
    return in_maps


def kernel(x, int_weight, weights_scales, reduced_w, fp_weight, bias,
           int_indices, fp_indices):
    global LAST_RESULTS
    from concourse.bass_utils import run_bass_kernel_spmd

    _ensure_ntff_hook()
    in_maps = _prep_inputs(x, int_weight, weights_scales, reduced_w,
                           fp_weight, bias, int_indices, fp_indices)
    nc = _get_program()
    res = run_bass_kernel_spmd(nc, in_maps, core_ids=list(range(NCORES)))
    LAST_RESULTS = res
    out = np.concatenate([res.results[c]["out"] for c in range(NCORES)], axis=0)
    out = out[None].astype(np.float16)
    b = np.asarray(bias).astype(np.float32)
    if np.any(b):
        # bias is all-zero in this problem's setup; handled here for
        # completeness without spending a matmul K-row on it
        out = (out.astype(np.float32) + b[None, None, :]).astype(np.float16)
    return out


# revision 48
# speedup vs baseline: 1.2208x; 1.0213x over previous
"""MixedQLinear Trainium2 kernel — token-parallel, merged-K version.

Computation (per reference):
  x2 = x[0]                                  (M=4096, IN_F=4096) fp16
  int_x = x2[:, int_indices]                 (M, 3840)
  fp_x  = x2[:, fp_indices]                  (M, 256)
  per-token asym quant of int_x to int4:  scale=(mx-mn)/15, zero=mn
  q = round((int_x-zero)/scale) - 8          in [-8,7]
  out = scale*w_scale*(q @ w_int.T) + (zero+8*scale)*reduced_w + fp_x@fp_w.T + bias

Strategy: shard TOKENS across the 8 cores (512 each); every core holds
the full out_features dimension.  No collective is needed.

Key algebra: divide the fp weights by wscale on the host and the fp
activations by scale_t on the device, then the fp path rides INSIDE the
int matmul (K = 3840 int + 256 fp = 4096 exactly = 16 fp8 DoubleRow
matmuls), because the combine multiplies the whole psum by
scale_t*wscale_o.  The zero-point term (mn+8*scale)*reduced_w becomes
(mn*rs+8) * (reduced_w/wscale) * (scale*wscale) and is pre-filled into
the PSUM bank by the ACT engine before the matmuls accumulate onto it.
Final combine is ONE fused DVE op: out = (p0*scale_t)*wscale_bcast.

Phase A per 128-token tile (split in 2 K-chunks for latency):
  - min (DVE fused tensor_tensor_reduce) / max (Pool) stats,
  - params: scale, rs=1/scale, bqC=-mn*rs-8+C, alphas=mn*rs+8,
  - quant+round in ONE ACT op: y = x*rs + (bqC) in f32 (magic constant
    C=1.5*2^23 forces RNE-to-integer), DVE subtracts C casting to f16,
  - fp cols: ACT copy with scale=rs into the same qa staging,
  - DMA xbar transpose (sync ring) -> k-major, DVE cast f16->f8.

DMA queues: weights (16.8MB) alone on the scalar HWDGE ring; the 8
transposes alone on the sync HWDGE ring; x tiles, consts and output
stores on the gpsimd SWDGE queue.

Host side does only layout work: column gather, int4 unpack, weight
merge + fp8 cast, broadcasts, token slicing, concat of outputs.
"""

import os
import sys

import numpy as np

for _p in ("/opt/trn_rl_repo",):
    if _p not in sys.path and os.path.isdir(_p):
        sys.path.insert(0, _p)

TOKENS = 4096
IN_F = 4096
OUT_F = 4096
FP_F = 256
INT_F = IN_F - FP_F          # 3840
NCORES = 8
TPC = TOKENS // NCORES       # 512 tokens per core
NT = TPC // 128              # 4 token tiles per core
KE = IN_F // 128             # 32 k-planes (30 int + 2 fp)
HKE = KE // 2                # 16 planes per half-chunk
NG = 8                       # out-feature groups per core
OG = OUT_F // NG             # 512 out features per group
C_MAGIC = 12582912.0         # 1.5*2^23: fp32 add/sub forces RNE-to-integer

# 'psum': ACT pre-fills the correction into the PSUM bank, matmuls
# accumulate onto it (start=False).  'sbuf': ACT writes the correction
# to SBUF and the combine adds it (2 DVE ops) with normal matmul groups.
PREFILL_MODE = "sbuf"

_PROGRAM = None
LAST_RESULTS = None


def _ensure_ntff_hook():
    """Install the axon NTFF profiling hook if the image's antenv lacks it.

    Best-effort: profiling only; compile/run work without it.
    """
    import contextlib
    import ctypes
    import types

    try:
        try:
            import antenv.axon_hooks as hooks_mod
        except ImportError:
            import antenv

            hooks_mod = types.ModuleType("antenv.axon_hooks")
            _holder = {}
            hooks_mod.set_axon_ntff_profile_hook = (
                lambda hook: _holder.__setitem__("hook", hook))
            hooks_mod.get_axon_ntff_profile_hook = (
                lambda: _holder.get("hook"))
            sys.modules["antenv.axon_hooks"] = hooks_mod
            antenv.axon_hooks = hooks_mod

        if hooks_mod.get_axon_ntff_profile_hook() is not None:
            return
        so_path = "/opt/axon/libaxon_pjrt.so"
        if not os.path.exists(so_path):
            return
        lib = ctypes.CDLL(so_path)
        if not hasattr(lib, "axon_start_nrt_profile"):
            return
        lib.axon_start_nrt_profile.argtypes = [
            ctypes.POINTER(ctypes.c_int64), ctypes.c_size_t]
        lib.axon_start_nrt_profile.restype = ctypes.c_int64
        lib.axon_stop_nrt_profile.argtypes = [ctypes.c_char_p]
        lib.axon_stop_nrt_profile.restype = ctypes.c_int64

        @contextlib.contextmanager
        def _hook(output_dir, device_ids):
            import jax

            jax.devices()
            if device_ids:
                ids = (ctypes.c_int64 * len(device_ids))(*device_ids)
                rc = lib.axon_start_nrt_profile(ids, len(device_ids))
            else:
                rc = lib.axon_start_nrt_profile(None, 0)
            if rc != 0:
                raise RuntimeError(f"axon_start_nrt_profile rc={rc}")
            try:
                yield
            finally:
                n = lib.axon_stop_nrt_profile(str(output_dir).encode())
                print(f"ntff profile: {n} file(s) written to {output_dir}")

        hooks_mod.set_axon_ntff_profile_hook(_hook)
    except Exception:
        pass


def _build_program():
    import concourse.mybir as mybir
    import concourse.tile as tile
    from concourse import bacc

    f16 = mybir.dt.float16
    f32 = mybir.dt.float32
    f8 = mybir.dt.float8e4
    Alu = mybir.AluOpType
    Act = mybir.ActivationFunctionType

    nc = bacc.Bacc(None, target_bir_lowering=False)

    # gathered x, token-major: cols 0..3839 int features, 3840..4095 fp
    xg_d = nc.dram_tensor("xg", [TPC, IN_F], f16, kind="ExternalInput")
    # merged weights, k-major fp8: wq[g,p,e,o] = Wm[e*128+p, g*512+o]
    # rows 0..3839 = raw int4 vals (fp8-exact), 3840..4095 = fp_w/wscale
    wq_d = nc.dram_tensor("wq", [NG, 128, KE, OG], f8, kind="ExternalInput")
    # wscale broadcast to 128 partitions
    wsb_d = nc.dram_tensor("wsb", [128, OUT_F], f16, kind="ExternalInput")
    # reduced_w/wscale broadcast to 128 partitions (corr prefill input)
    rwb_d = nc.dram_tensor("rwb", [128, OUT_F], f16, kind="ExternalInput")
    # identity for PE transposes
    id_d = nc.dram_tensor("ident", [128, 128], f16, kind="ExternalInput")
    out_d = nc.dram_tensor("out", [TPC, OUT_F], f16, kind="ExternalOutput")
    debug = bool(os.environ.get("KBG_DEBUG"))
    if debug:
        dbg_q8 = nc.dram_tensor("dbg_q8", [2, 128, HKE, 128], f8,
                                kind="ExternalOutput")
        dbg_qa = nc.dram_tensor("dbg_qa", [2, 128, 2048], f16,
                                kind="ExternalOutput")
        dbg_pp = nc.dram_tensor("dbg_pp", [128, 4 * NT], f32,
                                kind="ExternalOutput")
        dbg_m = nc.dram_tensor("dbg_m", [128, OG], f32,
                               kind="ExternalOutput")

    with tile.TileContext(nc) as tc:
        with tc.tile_pool(name="consts", bufs=1) as consts, \
             tc.tile_pool(name="xin", bufs=3) as xin, \
             tc.tile_pool(name="y0p", bufs=2) as y0p, \
             tc.tile_pool(name="qap", bufs=2) as qap, \
             tc.tile_pool(name="qt8", bufs=2 * NT) as qt8, \
             tc.tile_pool(name="wqp", bufs=5) as wqp, \
             tc.tile_pool(name="jnk", bufs=1) as jnk, \
             tc.tile_pool(name="stp", bufs=4) as stp, \
             tc.tile_pool(name="outp", bufs=4) as outp, \
             tc.tile_pool(name="corrp", bufs=4) as corrp, \
             tc.tile_pool(name="ps0", bufs=6, space="PSUM") as ps0, \
             tc.tile_pool(name="pstr", bufs=1, space="PSUM") as pstr:

            # --- sync HWDGE ring: x tiles (in half-K chunks so stats can
            # start on the first half while the second streams)
            xts = []
            for r in range(NT):
                xa = xin.tile([128, 2048], f16, tag="xa")
                nc.sync.dma_start(
                    out=xa[:, :], in_=xg_d[r * 128:(r + 1) * 128, :2048])
                xb = xin.tile([128, 2048], f16, tag="xb")
                nc.sync.dma_start(
                    out=xb[:, :], in_=xg_d[r * 128:(r + 1) * 128, 2048:])
                xts.append((xa, xb))

            # --- scalar HWDGE ring: weights only; first 4 groups queued
            # immediately so the ring never starves
            wq_tiles = []

            def load_wq(g, eng=None):
                wqg = wqp.tile([128, KE, OG], f8, name="wqg")
                eng = eng or nc.sync
                eng.dma_start(out=wqg[:, :, :], in_=wq_d[g, :, :, :])
                wq_tiles.append(wqg)

            # ALL weights ride the sync ring FIFO behind the x tiles: x
            # gets the full early HBM bandwidth, the ring never starves
            # (triggers all issue from the idle sync engine), and pool-slot
            # waits block nothing that matters
            load_wq(0)
            load_wq(1)

            # --- SWDGE: identity + broadcast consts (stores join later)
            id_s = consts.tile([128, 128], f16)
            nc.gpsimd.dma_start(out=id_s[:, :], in_=id_d[:, :])
            wsb_s = consts.tile([128, OUT_F], f16)
            nc.gpsimd.dma_start(out=wsb_s[:, :], in_=wsb_d[:, :])
            rwb_s = consts.tile([128, OUT_F], f16)
            nc.gpsimd.dma_start(out=rwb_s[:, :], in_=rwb_d[:, :])

            # [scale, rs, bq, alpha] packed per tile r at ppack[:, 4r:4r+4]
            ppack = consts.tile([128, 4 * NT], f32)

            def param(r, v):
                idx = 4 * r + v
                return ppack[:, idx:idx + 1]

            def stats_params(r):
                xa, xb = xts[r]
                mn = stp.tile([128, 1], f32, tag="mn")
                mx = stp.tile([128, 1], f32, tag="mx")
                mnx = [None] * 4
                # per-half min/max trees: half A runs while half B streams
                for hi, (xh, ilen) in enumerate(((xa, 2048), (xb, 1792))):
                    l1 = ilen // 2
                    for oi, op in enumerate((Alu.min, Alu.max)):
                        a1 = jnk.tile([128, 1024], f16, tag="a1")
                        a2 = jnk.tile([128, 512], f16, tag="a2")
                        pp = stp.tile([128, 1], f32, tag=f"pp{hi}{oi}")
                        nc.vector.tensor_tensor(
                            out=a1[:, :l1], in0=xh[:, :l1],
                            in1=xh[:, l1:ilen], op=op)
                        nc.vector.tensor_tensor(
                            out=a2[:, :l1 // 2], in0=a1[:, :l1 // 2],
                            in1=a1[:, l1 // 2:l1], op=op)
                        nc.vector.tensor_reduce(
                            out=pp[:, :], in_=a2[:, :l1 // 2],
                            axis=mybir.AxisListType.X, op=op)
                        mnx[2 * hi + oi] = pp
                nc.vector.tensor_tensor(
                    out=mn[:, :], in0=mnx[0][:, :], in1=mnx[2][:, :],
                    op=Alu.min)
                nc.vector.tensor_tensor(
                    out=mx[:, :], in0=mnx[1][:, :], in1=mnx[3][:, :],
                    op=Alu.max)
                hp = tc.high_priority()
                hp.__enter__()
                d = stp.tile([128, 1], f32, tag="d")
                nc.vector.tensor_sub(d[:, :], mx[:, :], mn[:, :])
                nc.vector.tensor_scalar(
                    out=param(r, 0), in0=d[:, :],
                    scalar1=1.0 / 15.0, scalar2=1e-8, op0=Alu.mult,
                    op1=Alu.max)
                nc.vector.reciprocal(param(r, 1), param(r, 0))
                tt = stp.tile([128, 1], f32, tag="tt")
                nc.vector.tensor_mul(tt[:, :], mn[:, :], param(r, 1))
                # bq = -mn*rs - 8
                nc.vector.tensor_scalar(
                    out=param(r, 2), in0=tt[:, :],
                    scalar1=-1.0, scalar2=-8.0, op0=Alu.mult,
                    op1=Alu.add)
                # alpha = mn + 8*scale
                t8 = stp.tile([128, 1], f32, tag="t8")
                nc.vector.tensor_scalar(
                    out=t8[:, :], in0=param(r, 0),
                    scalar1=8.0, scalar2=None, op0=Alu.mult)
                nc.vector.tensor_add(param(r, 3), t8[:, :], mn[:, :])
                hp.__exit__(None, None, None)

            q8s = [[None, None] for _ in range(NT)]

            def phase_a(r):
                """stats -> quant+round per half -> PE transpose -> cast."""
                stats_params(r)
                for h in (0, 1):
                    q8 = qt8.tile([128, HKE, 128], f8, tag=f"q8_{r}{h}",
                                  bufs=1)
                    q8s[r][h] = q8
                for h in (0, 1):
                    xh = xts[r][h]
                    ci = 2048 if h == 0 else INT_F - 2048
                    y0 = y0p.tile([128, 2048], f32, tag=f"y{h}")
                    nc.scalar.activation(
                        out=y0[:, :ci], in_=xh[:, :ci],
                        func=Act.Identity,
                        bias=param(r, 2), scale=param(r, 1))
                    qa = qap.tile([128, 2048], f16, tag=f"qa{h}")
                    # (y0+C)-C: fp32-internal RNE round to integer, f16 out
                    nc.vector.tensor_scalar(
                        out=qa[:, :ci], in0=y0[:, :ci], scalar1=C_MAGIC,
                        scalar2=-C_MAGIC, op0=Alu.add, op1=Alu.add)
                    if h == 1:
                        # fp cols: x_fp * rs (NOT rounded)
                        nc.scalar.activation(
                            out=qa[:, ci:], in_=xh[:, ci:],
                            func=Act.Identity, scale=param(r, 1))
                    # k-major via PE transpose, 2 chunks of 8 planes;
                    # ACT casts each psum chunk to fp8 in SBUF
                    for c in (0, 1):
                        pt = pstr.tile([128, 8, 128], f16, tag=f"pt{c}")
                        for j in range(8):
                            col = c * 1024 + j * 128
                            nc.tensor.transpose(
                                pt[:, j, :], qa[:, col:col + 128],
                                id_s[:, :])
                        nc.scalar.copy(
                            q8s[r][h][:, c * 8:(c + 1) * 8, :], pt[:, :, :])

            def block(g, t):
                wqg = wq_tiles[g]
                o0 = g * OG
                t0 = t * 128
                # corr = alpha_t * (redw/wscale)_o on ACT; rides the
                # combine as the affine_then_add in1
                cr = corrp.tile([128, OG], f16, name="cr")
                nc.scalar.activation(
                    out=cr[:, :], in_=rwb_s[:, o0:o0 + OG],
                    func=Act.Identity, scale=param(t, 3))
                p0 = ps0.tile([128, OG], f32, name="p0")
                for e in range(KE // 2):
                    h, el = (0, e) if e < HKE // 2 else (1, e - HKE // 2)
                    nc.tensor.matmul(
                        p0[:, :], q8s[t][h][:, 2 * el:2 * el + 2, :],
                        wqg[:, 2 * e:2 * e + 2, :],
                        start=(e == 0), stop=(e == KE // 2 - 1),
                        perf_mode=mybir.MatmulPerfMode.DoubleRow)
                # out = (p0*scale_t + cr/wscale-fold)*wscale
                #     = scale_t*wscale*p0 + alpha*redw
                m = outp.tile([128, OG], f32, tag="m")
                nc.vector.affine_then_add(
                    out=m[:, :], in0=p0[:, :], in1=cr[:, :],
                    scale=param(t, 0), bias=0.0)
                ot = outp.tile([128, OG], f16, tag="ot")
                nc.vector.tensor_mul(ot[:, :], m[:, :], wsb_s[:, o0:o0 + OG])
                nc.gpsimd.dma_start(
                    out=out_d[t0:t0 + 128, o0:o0 + OG], in_=ot[:, :])

            # cohorts of 2/3/3 groups, t-outer inside a cohort: early HBM
            # demand stays low (x + 2 weight groups), and later weight
            # triggers are placed right after the block that frees the
            # buffer they rotate into
            for g in range(2, NG):
                load_wq(g)
            phase_a(0)
            block(0, 0)
            phase_a(1)
            block(1, 0)
            block(0, 1)
            phase_a(2)
            block(1, 1)
            block(0, 2)
            phase_a(3)
            block(1, 2)
            block(0, 3)
            block(1, 3)
            for t in range(NT):
                for gg in (2, 3, 4):
                    block(gg, t)
            for t in range(NT):
                for gg in (5, 6, 7):
                    block(gg, t)

    nc.finalize()
    return nc


def _get_program():
    global _PROGRAM
    if _PROGRAM is None:
        _PROGRAM = _build_program()
    return _PROGRAM


def _unpack_i4(w_packed):
    """(out, INT_F//2) uint8 -> (out, INT_F) int8; col 2k=low nibble, 2k+1=high."""
    lo = (w_packed & 0x0F).astype(np.int8)
    hi = ((w_packed >> 4) & 0x0F).astype(np.int8)
    lo = np.where(lo >= 8, lo - 16, lo)
    hi = np.where(hi >= 8, hi - 16, hi)
    w = np.empty((w_packed.shape[0], w_packed.shape[1] * 2), dtype=np.int8)
    w[:, 0::2] = lo
    w[:, 1::2] = hi
    return w


def _prep_inputs(x, int_weight, weights_scales, reduced_w, fp_weight, bias,
                 int_indices, fp_indices):
    import ml_dtypes
    f8np = ml_dtypes.float8_e4m3

    x2 = np.asarray(x, dtype=np.float16)[0]
    int_idx = np.asarray(int_indices).astype(np.int64)
    fp_idx = np.asarray(fp_indices).astype(np.int64)

    # gathered x: int cols then fp cols, token-major
    all_idx = np.concatenate([int_idx, fp_idx])
    xg = np.ascontiguousarray(x2[:, all_idx])               # (M, 4096) f16

    wsc = np.asarray(weights_scales).astype(np.float32)     # (OUT_F, 1)
    w_int = _unpack_i4(np.asarray(int_weight))              # (OUT_F, 3840) int8
    fpW = np.asarray(fp_weight).astype(np.float32)          # (OUT_F, 256)
    # merged k-major weights: rows 0..3839 raw int4, 3840..4095 fp_w/wsc
    Wm = np.empty((IN_F, OUT_F), dtype=np.float32)
    Wm[:INT_F, :] = w_int.T
    Wm[INT_F:, :] = (fpW / wsc).T
    wq = np.ascontiguousarray(
        Wm.reshape(KE, 128, NG, OG).transpose(2, 1, 0, 3)).astype(f8np)

    wsb = np.broadcast_to(
        wsc[:, 0].astype(np.float16)[None, :], (128, OUT_F)).copy()
    redw = np.asarray(reduced_w).astype(np.float32)         # (1, OUT_F)
    rwb = np.broadcast_to(
        (redw[0] / wsc[:, 0]).astype(np.float16)[None, :], (128, OUT_F)
    ).copy()

    ident = np.eye(128, dtype=np.float16)
    in_maps = []
    for c in range(NCORES):
        tok = slice(c * TPC, (c + 1) * TPC)
        in_maps.append({"xg": xg[tok], "wq": wq, "wsb": wsb, "rwb": rwb,
                        "ident": ident})
    return in_maps


def kernel(x, int_weight, weights_scales, reduced_w, fp_weight, bias,
           int_indices, fp_indices):
    global LAST_RESULTS
    from concourse.bass_utils import run_bass_kernel_spmd

    _ensure_ntff_hook()
    in_maps = _prep_inputs(x, int_weight, weights_scales, reduced_w,
                           fp_weight, bias, int_indices, fp_indices)
    nc = _get_program()
    res = run_bass_kernel_spmd(nc, in_maps, core_ids=list(range(NCORES)))
    LAST_RESULTS = res
    out = np.concatenate([res.results[c]["out"] for c in range(NCORES)], axis=0)
    out = out[None].astype(np.float16)
    b = np.asarray(bias).astype(np.float32)
    if np.any(b):
        # bias is all-zero in this problem's setup; handled here for
        # completeness without spending a matmul K-row on it
        out = (out.astype(np.float32) + b[None, None, :]).astype(np.float16)
    return out
